# revision 36
# baseline (speedup 1.0000x reference)
"""Trainium2 Bass kernel for nn_Encoder_74182675137046.

Reference computation (per image of 1024x1024 complex pixels):
    feats = [norm_row, norm_col, x0, x1]  per pixel     [N, 4]
    h   = relu((feats @ W1 + b1) @ W2 + b2)             [N, 128]
    out = h @ W3 + b3                                   [N, 128]
    result = (w * out).sum(0) / w.sum()                 [128]
with w = (x0 != 0), and norm_row/col normalized by masked min/max.

Algebraic folding (exact):
    fc1+fc2 fold:  h_pre = feats @ W12 + b12,  W12 = W1@W2, b12 = b1@W2 + b2
    pool/fc3 swap: (w*out).sum = (sum_p w_p*relu(h_pre_p)) @ W3 + w.sum()*b3
So the device only computes S = sum_p relu(h_pre_p)  (a [128] vector per
core); the tiny [128]x[128,128] tail runs on host in float64.

Device design (per core, 128 image rows = 131072 points):
  - rhs features at partitions {32g+k}: x0, x1, norm_col, ones; the
    per-image-row bias (b12 + nr*W12[0]) is folded into lhsT row 3, one
    128-col weight block per (group, row-slot).  lhsT table is packed to
    16 partition rows in DRAM (256KB) and scattered on load.
  - fp32r matmuls, N=512, 4 tile_position row groups; 16 matmuls per fill
    (4 quarters x 4 groups) = 8192 points.
  - PSUM as a 4-deep rotation of 2-bank sets (FD=1024 each).  Each set is
    drained by ONE whole-set consumer (ScalarE relu+accum or VectorE
    max0+add+accum), owners strictly alternating so every PSUM WAR chain
    is same-engine.  The hardware allows ONE sem wait per instruction:
    same-engine deps are downgraded to nosync (engine FIFO provides the
    ordering), per-fill observer matmuls absorb the DMA-queue waits, and
    the 4-deep rotation gives the PE two consumer-durations of slack so
    matmuls never chase consumers.  All DMAs use single-partition-dim APs
    (the DGE mis-executes nested partition patterns) on the HWDGE ring.
    Measured: 104.9us (baseline 146.9us); VectorE is saturated steady-state.
"""

import numpy as np

import concourse.bass as bass
import concourse.tile as tile
from concourse import mybir
from concourse.bass_utils import run_bass_kernel_spmd
from concourse.tile_rust import add_dep_helper

H = 1024
W = 1024
D = 128
N_CORES = 8
ROWS_PER_CORE = H // N_CORES          # 128
NPTS = ROWS_PER_CORE * W              # 131072
CHUNK = 2 * W                         # 2048 pts per (group, fill) = 2 image rows
NGROUPS = 4
FILL_PTS = NGROUPS * CHUNK            # 8192
NFILLS = NPTS // FILL_PTS             # 16
NT = 512
ROW_SLOTS = 2 * NFILLS                # lhsT blocks per group (32)
NSETS = NFILLS * 8                    # 128 two-bank consumer sets

F32 = mybir.dt.float32
F32R = mybir.dt.float32r

TRACE = False
LAST_RESULT = None

_NC_CACHE = None

def _owner_schedule():
    # Strict alternation: owners[s] == owners[s-4] always, so every PSUM
    # WAR chain is same-engine (FIFO-ordered) and needs no semaphore.
    return ["A" if s % 2 == 0 else "D" for s in range(NSETS)]


def _build_bass():
    """Build the SPMD Bass program (same program on all 8 cores)."""
    global _NC_CACHE
    if _NC_CACHE is not None:
        return _NC_CACHE

    nc = bass.Bass()

    xd = nc.dram_tensor("xd", [NFILLS, NGROUPS, 2, CHUNK], F32R,
                        kind="ExternalInput")
    lwt = nc.dram_tensor("lwt", [NGROUPS, 4, 128 * ROW_SLOTS], F32R,
                         kind="ExternalInput")
    ncpre = nc.dram_tensor("ncpre", [NGROUPS, 2 * CHUNK], F32R,
                           kind="ExternalInput")
    onepre = nc.dram_tensor("onepre", [NGROUPS, 2 * CHUNK], F32R,
                            kind="ExternalInput")
    outs = nc.dram_tensor("outs", [128, 1], F32, kind="ExternalOutput")
    import os
    dbg = os.environ.get("KDEBUG_SIM") == "1"
    if dbg:
        rinit = nc.dram_tensor("rinit", [128, NFILLS * CHUNK], F32R,
                               kind="ExternalInput")
        winit = nc.dram_tensor("winit", [128, 128 * ROW_SLOTS], F32R,
                               kind="ExternalInput")
        redout = nc.dram_tensor("redout", [128, NSETS], F32,
                                kind="ExternalOutput")

    owners = _owner_schedule()

    with tile.TileContext(nc) as tc:
        with (
            tc.tile_pool(name="singles", bufs=1) as singles,
            tc.tile_pool(name="psall", bufs=1, space="PSUM") as psall,
        ):
            lw_t = singles.tile([128, 128 * ROW_SLOTS], F32R)
            lw_g = lw_t.rearrange("(g r) c -> g r c", r=32)[:, 0:4, :]
            # Packed weight table: fills 0-3's blocks first so compute
            # starts early; the tail streams behind it.
            c0 = 128 * 8
            all_dmas = []
            # One DMA per lhsT row so every transfer has a single (strided)
            # partition dim -- the DGE mis-handles nested partition APs.
            # On the HWDGE sync ring: the SWDGE path delivers these ~10x
            # slower and gated compute start by ~7us.  Chunk0 (fills 0-3)
            # leads the ring; the tail is emitted at fill 1.
            for r in range(4):
                all_dmas.append(nc.sync.dma_start(
                    out=lw_g[:, r, 0:c0], in_=lwt[:, r, 0:c0]))

            red = singles.tile([128, NSETS], F32)
            rhs_all = singles.tile([128, NFILLS, CHUNK], F32R)
            # 4-deep rotation of 2-bank sets.
            ps = psall.tile([128, 4, 2, NT], F32)
            if dbg:
                all_dmas.append(nc.sync.dma_start(
                    out=rhs_all.rearrange("p f c -> p (f c)"), in_=rinit[:]))
            rall = rhs_all.rearrange("(g r) s c -> g r (s c)", r=32)

            # Dep bookkeeping: the hardware allows ONE sem wait per
            # instruction.  Same-engine deps are guaranteed by the engine's
            # in-order FIFO, so downgrade them from sync (sem wait) to
            # nosync (scheduler-order only).  Cross-engine deps stay sync.
            inst_by_name = {}
            engine_of = {}

            def track(bi, eng):
                inst_by_name[bi.ins.name] = bi.ins
                engine_of[bi.ins.name] = eng
                return bi

            def downgrade_same_engine(bi, eng):
                for dep in list(bi.ins.sync_dependency_names()):
                    if engine_of.get(dep) == eng:
                        bi.ins.try_remove_dependency(dep)
                        add_dep_helper(bi.ins, inst_by_name[dep], sync=False,
                                       reason="same-engine FIFO")
                return bi

            cons_insts = []
            # HAM warm-up: dep-free dummy matmuls keep the PE activity
            # window hot while the first fill's DMAs are in flight, so
            # fill 0 streams at 2.4GHz instead of the cold 1.2GHz.  They
            # read garbage (red is uninitialized) and write a PSUM strip
            # the first real matmul overwrites.
            warm_cell = ps[0:1, 0, 0, 0:128]
            for _ in range(40):
                wm = nc.tensor.matmul(
                    warm_cell, red[0:4, 0:1].bitcast(F32),
                    red[0:4, 0:128].bitcast(F32),
                    start=True, stop=True, tile_position=(0, 0),
                )
                track(wm, "PE")
                downgrade_same_engine(wm, "PE")
            if dbg:
                # Absorb the rinit DMA's sem on a dedicated obs (reads a row
                # no real DMA touches) so later obs keep single waits.
                dsrc = rhs_all[0:5, 0, 0:1].bitcast(F32)
                dob = nc.tensor.matmul(ps[0:1, 0, 0, 0:1], dsrc, dsrc,
                                       start=True, stop=True,
                                       tile_position=(0, 0))
                track(dob, "PE")
                downgrade_same_engine(dob, "PE")
            sidx = 0
            for f in range(NFILLS):
                rhs = rhs_all[:, f, :]
                rg = rhs.rearrange("(g r) c -> g r c", r=32)

                # Constant-row prefills for (f, f+1) lead the ring, then
                # the fill's x0/x1 rows (one DMA per feature row; fresh
                # slots -> dep-free).
                if f % 2 == 0:
                    cols = slice(CHUNK * f, CHUNK * (f + 2))
                    all_dmas.append(
                        nc.sync.dma_start(out=rall[:, 2, cols], in_=ncpre[:]))
                    all_dmas.append(
                        nc.sync.dma_start(out=rall[:, 3, cols], in_=onepre[:]))
                if f == 1:
                    for r in range(4):
                        all_dmas.append(nc.sync.dma_start(
                            out=lw_g[:, r, c0:], in_=lwt[:, r, c0:]))
                for k in range(2):
                    all_dmas.append(
                        nc.sync.dma_start(out=rg[:, k, :], in_=xd[f, :, k]))

                # Observer cascade: tiny matmuls into a cell of the bank the
                # fill's first matmul will clear.  The first obs takes that
                # bank's WAR (on consumer(s-4), cross-engine); each later obs
                # reads one cell of a freshly-DMA'd region and thereby
                # absorbs that DMA queue's sem wait.  Real matmuls then get
                # their DMA waits elided (same engine, same sem, same count)
                # and carry only their own WAR wait.
                obs_cell = ps[0:1, 0, 0, 0:1]
                srcs = []
                if f == 0:
                    srcs += [lw_t[0 : r + 1, 0:1].bitcast(F32)
                             for r in range(4)]
                else:
                    srcs.append(lw_t[0:1, 0:1].bitcast(F32))
                srcs.append(rhs[0:1, 0:1].bitcast(F32))
                srcs.append(rhs[0:2, 0:1].bitcast(F32))
                if f % 2 == 0:
                    srcs.append(rhs[0:3, 0:1].bitcast(F32))
                    srcs.append(rhs[0:4, 0:1].bitcast(F32))
                if f == 4:
                    srcs += [lw_t[0 : r + 1, 128 * 8 : 128 * 8 + 1].bitcast(F32)
                             for r in range(4)]
                last_obs = None
                for src in srcs:
                    ob = nc.tensor.matmul(obs_cell, src, src, start=True,
                                          stop=True, tile_position=(0, 0))
                    track(ob, "PE")
                    downgrade_same_engine(ob, "PE")
                    last_obs = ob

                for q in range(4):
                    s_w = 2 * f + (q // 2)
                    col0 = q * NT
                    wcols = slice(128 * s_w, 128 * (s_w + 1))
                    for j in range(2):          # two banks per set
                        slot = sidx % 4
                        for g in (2 * j, 2 * j + 1):
                            mm = nc.tensor.matmul(
                                ps[:, slot, g - 2 * j, :],
                                lw_t[32 * g : 32 * g + 4, wcols],
                                rhs[32 * g : 32 * g + 4, col0 : col0 + NT],
                                start=True, stop=True,
                                tile_position=(32 * g, 0),
                            )
                            track(mm, "PE")
                            downgrade_same_engine(mm, "PE")
                            last_mm = mm
                            # Keep every matmul behind the fill's observers
                            # in the PE queue so the scheduler can't hoist
                            # one ahead and have it inherit a DMA wait on
                            # top of its WAR wait.
                            add_dep_helper(mm.ins, last_obs.ins, sync=False,
                                           reason="after obs")
                        if owners[sidx] == "A":
                            cons = nc.scalar.activation(
                                out=ps[:, slot, :, :],
                                in_=ps[:, slot, :, :],
                                func=mybir.ActivationFunctionType.Relu,
                                accum_out=red[:, sidx : sidx + 1],
                            )
                            track(cons, "ACT")
                            downgrade_same_engine(cons, "ACT")
                        else:
                            cons = nc.vector.tensor_scalar(
                                out=ps[:, slot, :, :],
                                in0=ps[:, slot, :, :],
                                scalar1=0.0,
                                scalar2=None,
                                op0=mybir.AluOpType.max,
                                op1=mybir.AluOpType.add,
                                accum_out=red[:, sidx : sidx + 1],
                            )
                            track(cons, "DVE")
                            downgrade_same_engine(cons, "DVE")
                        cons_insts.append(cons)
                        sidx += 1

            outs_t = singles.tile([128, 1], F32)
            rsum = nc.vector.reduce_sum(outs_t[:], red[:],
                                        axis=mybir.AxisListType.X)
            track(rsum, "DVE")
            downgrade_same_engine(rsum, "DVE")
            # SP bridge takes the cross-engine wait; the out-DMA then only
            # carries its sem-recycle wait (one per instruction max).
            br = nc.sync.drain(fusable=False)
            add_dep_helper(br.ins, rsum.ins, reason="bridge rsum")
            odma = nc.sync.dma_start(out=outs[:], in_=outs_t[:])
            # The bridge already waited DVE>=rsum; the SP sequencer enqueues
            # this DMA only after the bridge, so the direct dep needs no sem.
            for dep in list(odma.ins.sync_dependency_names()):
                if engine_of.get(dep) == "DVE":
                    odma.ins.try_remove_dependency(dep)
            add_dep_helper(odma.ins, br.ins, sync=False, reason="after bridge")
            if dbg:
                rdma = nc.sync.dma_start(out=redout[:], in_=red[:])
                for dep in list(rdma.ins.sync_dependency_names()):
                    if engine_of.get(dep) in ("ACT", "DVE"):
                        rdma.ins.try_remove_dependency(dep)
                add_dep_helper(rdma.ins, br.ins, sync=False,
                               reason="after bridge")
                all_dmas.append(rdma)

            # Pre-observe every proc on SP so the TileContext-exit drain
            # (single-wait NOP) has nothing left to wait on.
            drain_deps = [cons_insts[-2].ins, cons_insts[-1].ins, rsum.ins,
                          last_mm.ins, odma.ins]
            drain_deps += [d.ins for d in all_dmas]
            for dins in drain_deps:
                dr = nc.sync.drain(fusable=False)
                add_dep_helper(dr.ins, dins, reason="pre-drain observe")

    _NC_CACHE = nc
    return nc


def kernel(x, W1, b1, W2, b2, W3, b3):
    global LAST_RESULT
    x = np.asarray(x, dtype=np.float32)
    W1 = np.asarray(W1, dtype=np.float32)
    b1 = np.asarray(b1, dtype=np.float32)
    W2 = np.asarray(W2, dtype=np.float32)
    b2 = np.asarray(b2, dtype=np.float32)
    W3 = np.asarray(W3, dtype=np.float32)
    b3 = np.asarray(b3, dtype=np.float32)

    x0, x1 = x[0], x[1]
    mask = x0 != 0.0

    rows_any = mask.any(axis=1)
    cols_any = mask.any(axis=0)
    ridx = np.nonzero(rows_any)[0]
    cidx = np.nonzero(cols_any)[0]
    rmin, rmax = float(ridx[0]), float(ridx[-1])
    cmin, cmax = float(cidx[0]), float(cidx[-1])

    W12 = W1.astype(np.float64) @ W2.astype(np.float64)
    b12 = b1.astype(np.float64) @ W2.astype(np.float64) + b2
    v0 = W12[0]

    nr_all = (np.arange(H, dtype=np.float64) - rmin) / (rmax - rmin)
    nc_all = (np.arange(W, dtype=np.float64) - cmin) / (cmax - cmin)

    nc2 = np.tile(nc_all.astype(np.float32), 2)
    v2f = W12[2].astype(np.float32)
    v3f = W12[3].astype(np.float32)
    v1f = W12[1].astype(np.float32)

    ncpre = np.broadcast_to(
        np.tile(nc2, 2)[None, :], (NGROUPS, 2 * CHUNK)
    ).copy()
    onepre = np.ones((NGROUPS, 2 * CHUNK), dtype=np.float32)

    nc_prog = _build_bass()
    in_maps = []
    for c in range(N_CORES):
        shard = x[:, c * ROWS_PER_CORE : (c + 1) * ROWS_PER_CORE, :]
        sh = shard.reshape(2, NFILLS, NGROUPS, CHUNK)
        xdv = np.ascontiguousarray(sh.transpose(1, 2, 0, 3))

        btab = (
            b12[:, None]
            + np.outer(v0, nr_all[c * ROWS_PER_CORE : (c + 1) * ROWS_PER_CORE])
        ).astype(np.float32)
        lwtv = np.zeros((NGROUPS, 4, 128 * ROW_SLOTS), dtype=np.float32)
        for g in range(NGROUPS):
            for s in range(ROW_SLOTS):
                f, half = divmod(s, 2)
                r_loc = 8 * f + 2 * g + half
                blk = slice(128 * s, 128 * (s + 1))
                lwtv[g, 0, blk] = v2f
                lwtv[g, 1, blk] = v3f
                lwtv[g, 2, blk] = v1f
                lwtv[g, 3, blk] = btab[:, r_loc]
        im = {"xd": xdv, "lwt": lwtv, "ncpre": ncpre, "onepre": onepre}
        import os as _os
        if _os.environ.get("KDEBUG_SIM") == "1":
            im["rinit"] = np.zeros((128, NFILLS * CHUNK), np.float32)
            im["winit"] = np.zeros((128, 128 * ROW_SLOTS), np.float32)
        in_maps.append(im)

    res = run_bass_kernel_spmd(
        nc_prog, in_maps, core_ids=list(range(N_CORES)), trace=TRACE
    )
    LAST_RESULT = res

    S = np.zeros(D, dtype=np.float64)
    for c in range(N_CORES):
        S += res.results[c]["outs"][:, 0].astype(np.float64)

    if not mask.all():
        zr, zc = np.nonzero(~mask)
        hz = (
            np.outer(nr_all[zr], W12[0])
            + np.outer(nc_all[zc], W12[1])
            + np.outer(x1[zr, zc].astype(np.float64), W12[3])
            + b12[None, :]
        )
        S -= np.maximum(hz, 0.0).sum(axis=0)

    wsum = float(mask.sum())
    out = (S @ W3.astype(np.float64)) / wsum + b3.astype(np.float64)
    return out.astype(np.float32)


# revision 37
# speedup vs baseline: 1.0960x; 1.0960x over previous
"""Trainium2 Bass kernel for nn_Encoder_74182675137046.

Reference computation (per image of 1024x1024 complex pixels):
    feats = [norm_row, norm_col, x0, x1]  per pixel     [N, 4]
    h   = relu((feats @ W1 + b1) @ W2 + b2)             [N, 128]
    out = h @ W3 + b3                                   [N, 128]
    result = (w * out).sum(0) / w.sum()                 [128]
with w = (x0 != 0), and norm_row/col normalized by masked min/max.

Algebraic folding (exact):
    fc1+fc2 fold:  h_pre = feats @ W12 + b12,  W12 = W1@W2, b12 = b1@W2 + b2
    pool/fc3 swap: (w*out).sum = (sum_p w_p*relu(h_pre_p)) @ W3 + w.sum()*b3
So the device only computes S = sum_p relu(h_pre_p)  (a [128] vector per
core); the tiny [128]x[128,128] tail runs on host in float64.

Device design (per core, 128 image rows = 131072 points):
  - rhs features at partitions {32g+k}: x0, x1, norm_col, ones; the
    per-image-row bias (b12 + nr*W12[0]) is folded into lhsT row 3, one
    128-col weight block per (group, row-slot).  lhsT table is packed to
    16 partition rows in DRAM (256KB) and scattered on load.
  - fp32r matmuls, N=512, 4 tile_position row groups; 16 matmuls per fill
    (4 quarters x 4 groups) = 8192 points.
  - PSUM as a 4-deep rotation of 2-bank sets (FD=1024 each).  Each set is
    drained by ONE whole-set consumer (ScalarE relu+accum or VectorE
    max0+add+accum), owners strictly alternating so every PSUM WAR chain
    is same-engine.  The hardware allows ONE sem wait per instruction:
    same-engine deps are downgraded to nosync (engine FIFO provides the
    ordering), per-fill observer matmuls absorb the DMA-queue waits, and
    the 4-deep rotation gives the PE two consumer-durations of slack so
    matmuls never chase consumers.  All DMAs use single-partition-dim APs
    (the DGE mis-executes nested partition patterns) on the HWDGE ring.
    Measured: 104.9us (baseline 146.9us); VectorE is saturated steady-state.
"""

import numpy as np

import concourse.bass as bass
import concourse.tile as tile
from concourse import mybir
from concourse.bass_utils import run_bass_kernel_spmd
from concourse.tile_rust import add_dep_helper

H = 1024
W = 1024
D = 128
N_CORES = 8
ROWS_PER_CORE = H // N_CORES          # 128
NPTS = ROWS_PER_CORE * W              # 131072
CHUNK = 2 * W                         # 2048 pts per (group, fill) = 2 image rows
NGROUPS = 4
FILL_PTS = NGROUPS * CHUNK            # 8192
NFILLS = NPTS // FILL_PTS             # 16
NT = 512
ROW_SLOTS = 2 * NFILLS                # lhsT blocks per group (32)
NSETS = NFILLS * 8                    # 128 two-bank consumer sets

F32 = mybir.dt.float32
F32R = mybir.dt.float32r

TRACE = False
LAST_RESULT = None

_NC_CACHE = None

def _owner_schedule():
    # Strict alternation: owners[s] == owners[s-4] always, so every PSUM
    # WAR chain is same-engine (FIFO-ordered) and needs no semaphore.
    return ["A" if s % 2 == 0 else "D" for s in range(NSETS)]


def _build_bass():
    """Build the SPMD Bass program (same program on all 8 cores)."""
    global _NC_CACHE
    if _NC_CACHE is not None:
        return _NC_CACHE

    nc = bass.Bass()

    xd = nc.dram_tensor("xd", [NFILLS, NGROUPS, 2, CHUNK], F32R,
                        kind="ExternalInput")
    lwt = nc.dram_tensor("lwt", [NGROUPS, 4, 128 * ROW_SLOTS], F32R,
                         kind="ExternalInput")
    ncpre = nc.dram_tensor("ncpre", [NGROUPS, 2 * CHUNK], F32R,
                           kind="ExternalInput")
    onepre = nc.dram_tensor("onepre", [NGROUPS, 2 * CHUNK], F32R,
                            kind="ExternalInput")
    outs = nc.dram_tensor("outs", [128, 1], F32, kind="ExternalOutput")
    import os
    dbg = os.environ.get("KDEBUG_SIM") == "1"
    if dbg:
        rinit = nc.dram_tensor("rinit", [128, NFILLS * CHUNK], F32R,
                               kind="ExternalInput")
        winit = nc.dram_tensor("winit", [128, 128 * ROW_SLOTS], F32R,
                               kind="ExternalInput")
        redout = nc.dram_tensor("redout", [128, NSETS], F32,
                                kind="ExternalOutput")

    owners = _owner_schedule()

    with tile.TileContext(nc) as tc:
        with (
            tc.tile_pool(name="singles", bufs=1) as singles,
            tc.tile_pool(name="psall", bufs=1, space="PSUM") as psall,
        ):
            lw_t = singles.tile([128, 128 * ROW_SLOTS], F32R)
            lw_g = lw_t.rearrange("(g r) c -> g r c", r=32)[:, 0:4, :]
            # Packed weight table: fills 0-3's blocks first so compute
            # starts early; the tail streams behind it.
            c0 = 128 * 8
            all_dmas = []
            # One DMA per lhsT row so every transfer has a single (strided)
            # partition dim -- the DGE mis-handles nested partition APs.
            # On the HWDGE sync ring: the SWDGE path delivers these ~10x
            # slower and gated compute start by ~7us.  Chunk0 (fills 0-3)
            # leads the ring; the tail is emitted at fill 1.
            for r in range(4):
                all_dmas.append(nc.sync.dma_start(
                    out=lw_g[:, r, 0:c0], in_=lwt[:, r, 0:c0]))

            red = singles.tile([128, NSETS], F32)
            rhs_all = singles.tile([128, NFILLS, CHUNK], F32R)
            # 4-deep rotation of 2-bank sets.
            ps = psall.tile([128, 4, 2, NT], F32)
            if dbg:
                all_dmas.append(nc.sync.dma_start(
                    out=rhs_all.rearrange("p f c -> p (f c)"), in_=rinit[:]))
            rall = rhs_all.rearrange("(g r) s c -> g r (s c)", r=32)

            # Dep bookkeeping: the hardware allows ONE sem wait per
            # instruction.  Same-engine deps are guaranteed by the engine's
            # in-order FIFO, so downgrade them from sync (sem wait) to
            # nosync (scheduler-order only).  Cross-engine deps stay sync.
            inst_by_name = {}
            engine_of = {}

            def track(bi, eng):
                inst_by_name[bi.ins.name] = bi.ins
                engine_of[bi.ins.name] = eng
                return bi

            def downgrade_same_engine(bi, eng):
                for dep in list(bi.ins.sync_dependency_names()):
                    if engine_of.get(dep) == eng:
                        bi.ins.try_remove_dependency(dep)
                        add_dep_helper(bi.ins, inst_by_name[dep], sync=False,
                                       reason="same-engine FIFO")
                return bi

            cons_insts = []
            if dbg:
                # Absorb the rinit DMA's sem on a dedicated obs (reads a row
                # no real DMA touches) so later obs keep single waits.
                dsrc = rhs_all[0:5, 0, 0:1].bitcast(F32)
                dob = nc.tensor.matmul(ps[0:1, 0, 0, 0:1], dsrc, dsrc,
                                       start=True, stop=True,
                                       tile_position=(0, 0))
                track(dob, "PE")
                downgrade_same_engine(dob, "PE")
            sidx = 0
            for f in range(NFILLS):
                rhs = rhs_all[:, f, :]
                rg = rhs.rearrange("(g r) c -> g r c", r=32)

                # Constant-row prefills for (f, f+1) lead the ring, then
                # the fill's x0/x1 rows (one DMA per feature row; fresh
                # slots -> dep-free).
                if f % 2 == 0:
                    cols = slice(CHUNK * f, CHUNK * (f + 2))
                    all_dmas.append(
                        nc.sync.dma_start(out=rall[:, 2, cols], in_=ncpre[:]))
                    all_dmas.append(
                        nc.sync.dma_start(out=rall[:, 3, cols], in_=onepre[:]))
                if f == 1:
                    for r in range(4):
                        all_dmas.append(nc.sync.dma_start(
                            out=lw_g[:, r, c0:], in_=lwt[:, r, c0:]))
                for k in range(2):
                    all_dmas.append(
                        nc.sync.dma_start(out=rg[:, k, :], in_=xd[f, :, k]))

                # Observer cascade: tiny matmuls into a cell of the bank the
                # fill's first matmul will clear.  The first obs takes that
                # bank's WAR (on consumer(s-4), cross-engine); each later obs
                # reads one cell of a freshly-DMA'd region and thereby
                # absorbs that DMA queue's sem wait.  Real matmuls then get
                # their DMA waits elided (same engine, same sem, same count)
                # and carry only their own WAR wait.
                obs_cell = ps[0:1, 0, 0, 0:1]
                srcs = []
                if f == 0:
                    srcs += [lw_t[0 : r + 1, 0:1].bitcast(F32)
                             for r in range(4)]
                else:
                    srcs.append(lw_t[0:1, 0:1].bitcast(F32))
                srcs.append(rhs[0:1, 0:1].bitcast(F32))
                srcs.append(rhs[0:2, 0:1].bitcast(F32))
                if f % 2 == 0:
                    srcs.append(rhs[0:3, 0:1].bitcast(F32))
                    srcs.append(rhs[0:4, 0:1].bitcast(F32))
                if f == 4:
                    srcs += [lw_t[0 : r + 1, 128 * 8 : 128 * 8 + 1].bitcast(F32)
                             for r in range(4)]
                last_obs = None
                for src in srcs:
                    ob = nc.tensor.matmul(obs_cell, src, src, start=True,
                                          stop=True, tile_position=(0, 0))
                    track(ob, "PE")
                    downgrade_same_engine(ob, "PE")
                    last_obs = ob

                for q in range(4):
                    s_w = 2 * f + (q // 2)
                    col0 = q * NT
                    wcols = slice(128 * s_w, 128 * (s_w + 1))
                    for j in range(2):          # two banks per set
                        slot = sidx % 4
                        for g in (2 * j, 2 * j + 1):
                            mm = nc.tensor.matmul(
                                ps[:, slot, g - 2 * j, :],
                                lw_t[32 * g : 32 * g + 4, wcols],
                                rhs[32 * g : 32 * g + 4, col0 : col0 + NT],
                                start=True, stop=True,
                                tile_position=(32 * g, 0),
                            )
                            track(mm, "PE")
                            downgrade_same_engine(mm, "PE")
                            last_mm = mm
                            # Keep every matmul behind the fill's observers
                            # in the PE queue so the scheduler can't hoist
                            # one ahead and have it inherit a DMA wait on
                            # top of its WAR wait.
                            add_dep_helper(mm.ins, last_obs.ins, sync=False,
                                           reason="after obs")
                        if owners[sidx] == "A":
                            cons = nc.scalar.activation(
                                out=ps[:, slot, :, :],
                                in_=ps[:, slot, :, :],
                                func=mybir.ActivationFunctionType.Relu,
                                accum_out=red[:, sidx : sidx + 1],
                            )
                            track(cons, "ACT")
                            downgrade_same_engine(cons, "ACT")
                        else:
                            cons = nc.vector.tensor_scalar(
                                out=ps[:, slot, :, :],
                                in0=ps[:, slot, :, :],
                                scalar1=0.0,
                                scalar2=None,
                                op0=mybir.AluOpType.max,
                                op1=mybir.AluOpType.add,
                                accum_out=red[:, sidx : sidx + 1],
                            )
                            track(cons, "DVE")
                            downgrade_same_engine(cons, "DVE")
                        cons_insts.append(cons)
                        sidx += 1

            outs_t = singles.tile([128, 1], F32)
            rsum = nc.vector.reduce_sum(outs_t[:], red[:],
                                        axis=mybir.AxisListType.X)
            track(rsum, "DVE")
            downgrade_same_engine(rsum, "DVE")
            # SP bridge takes the cross-engine wait; the out-DMA then only
            # carries its sem-recycle wait (one per instruction max).
            br = nc.sync.drain(fusable=False)
            add_dep_helper(br.ins, rsum.ins, reason="bridge rsum")
            odma = nc.sync.dma_start(out=outs[:], in_=outs_t[:])
            # The bridge already waited DVE>=rsum; the SP sequencer enqueues
            # this DMA only after the bridge, so the direct dep needs no sem.
            for dep in list(odma.ins.sync_dependency_names()):
                if engine_of.get(dep) == "DVE":
                    odma.ins.try_remove_dependency(dep)
            add_dep_helper(odma.ins, br.ins, sync=False, reason="after bridge")
            if dbg:
                rdma = nc.sync.dma_start(out=redout[:], in_=red[:])
                for dep in list(rdma.ins.sync_dependency_names()):
                    if engine_of.get(dep) in ("ACT", "DVE"):
                        rdma.ins.try_remove_dependency(dep)
                add_dep_helper(rdma.ins, br.ins, sync=False,
                               reason="after bridge")
                all_dmas.append(rdma)

            # Pre-observe every proc on SP so the TileContext-exit drain
            # (single-wait NOP) has nothing left to wait on.
            drain_deps = [cons_insts[-2].ins, cons_insts[-1].ins, rsum.ins,
                          last_mm.ins, odma.ins]
            drain_deps += [d.ins for d in all_dmas]
            for dins in drain_deps:
                dr = nc.sync.drain(fusable=False)
                add_dep_helper(dr.ins, dins, reason="pre-drain observe")

    _NC_CACHE = nc
    return nc


def kernel(x, W1, b1, W2, b2, W3, b3):
    global LAST_RESULT
    x = np.asarray(x, dtype=np.float32)
    W1 = np.asarray(W1, dtype=np.float32)
    b1 = np.asarray(b1, dtype=np.float32)
    W2 = np.asarray(W2, dtype=np.float32)
    b2 = np.asarray(b2, dtype=np.float32)
    W3 = np.asarray(W3, dtype=np.float32)
    b3 = np.asarray(b3, dtype=np.float32)

    x0, x1 = x[0], x[1]
    mask = x0 != 0.0

    rows_any = mask.any(axis=1)
    cols_any = mask.any(axis=0)
    ridx = np.nonzero(rows_any)[0]
    cidx = np.nonzero(cols_any)[0]
    rmin, rmax = float(ridx[0]), float(ridx[-1])
    cmin, cmax = float(cidx[0]), float(cidx[-1])

    W12 = W1.astype(np.float64) @ W2.astype(np.float64)
    b12 = b1.astype(np.float64) @ W2.astype(np.float64) + b2
    v0 = W12[0]

    nr_all = (np.arange(H, dtype=np.float64) - rmin) / (rmax - rmin)
    nc_all = (np.arange(W, dtype=np.float64) - cmin) / (cmax - cmin)

    nc2 = np.tile(nc_all.astype(np.float32), 2)
    v2f = W12[2].astype(np.float32)
    v3f = W12[3].astype(np.float32)
    v1f = W12[1].astype(np.float32)

    ncpre = np.broadcast_to(
        np.tile(nc2, 2)[None, :], (NGROUPS, 2 * CHUNK)
    ).copy()
    onepre = np.ones((NGROUPS, 2 * CHUNK), dtype=np.float32)

    nc_prog = _build_bass()
    in_maps = []
    for c in range(N_CORES):
        shard = x[:, c * ROWS_PER_CORE : (c + 1) * ROWS_PER_CORE, :]
        sh = shard.reshape(2, NFILLS, NGROUPS, CHUNK)
        xdv = np.ascontiguousarray(sh.transpose(1, 2, 0, 3))

        btab = (
            b12[:, None]
            + np.outer(v0, nr_all[c * ROWS_PER_CORE : (c + 1) * ROWS_PER_CORE])
        ).astype(np.float32)
        lwtv = np.zeros((NGROUPS, 4, 128 * ROW_SLOTS), dtype=np.float32)
        for g in range(NGROUPS):
            for s in range(ROW_SLOTS):
                f, half = divmod(s, 2)
                r_loc = 8 * f + 2 * g + half
                blk = slice(128 * s, 128 * (s + 1))
                lwtv[g, 0, blk] = v2f
                lwtv[g, 1, blk] = v3f
                lwtv[g, 2, blk] = v1f
                lwtv[g, 3, blk] = btab[:, r_loc]
        im = {"xd": xdv, "lwt": lwtv, "ncpre": ncpre, "onepre": onepre}
        import os as _os
        if _os.environ.get("KDEBUG_SIM") == "1":
            im["rinit"] = np.zeros((128, NFILLS * CHUNK), np.float32)
            im["winit"] = np.zeros((128, 128 * ROW_SLOTS), np.float32)
        in_maps.append(im)

    res = run_bass_kernel_spmd(
        nc_prog, in_maps, core_ids=list(range(N_CORES)), trace=TRACE
    )
    LAST_RESULT = res

    S = np.zeros(D, dtype=np.float64)
    for c in range(N_CORES):
        S += res.results[c]["outs"][:, 0].astype(np.float64)

    if not mask.all():
        zr, zc = np.nonzero(~mask)
        hz = (
            np.outer(nr_all[zr], W12[0])
            + np.outer(nc_all[zc], W12[1])
            + np.outer(x1[zr, zc].astype(np.float64), W12[3])
            + b12[None, :]
        )
        S -= np.maximum(hz, 0.0).sum(axis=0)

    wsum = float(mask.sum())
    out = (S @ W3.astype(np.float64)) / wsum + b3.astype(np.float64)
    return out.astype(np.float32)
